# revision 100
# baseline (speedup 1.0000x reference)
"""Multi-head attention (B=2, S=2048, D=1024, H=16) on 8 Trainium2 cores.

Sharding: pure tensor-parallel over heads (Megatron): core c owns heads
{2c, 2c+1} (d_local = 128 columns of Wq/Wk/Wv, 128 rows of Wo) and
processes BOTH batches. Each core emits a [2, 2048, 1024] partial output
(row-parallel Wo); the host sums the 8 partials per batch.

Why: the SPMD program's attention work scales with KT0+KT1 (per 2 heads)
instead of 4*max(KT0,KT1) (per 4 heads) under the old batch x head-group
split, so key-length imbalance between the two batches no longer inflates
the program's critical path.

Key-side truncation: only ceil(valid_len/128) key tiles per batch are
computed; the per-batch mask rides the ScalarE exp as a per-partition
bias. Scores are computed transposed ([key, query]); the softmax
denominator comes free via 64 ones-columns appended to V (ones-trick).

Precision: fp16 streams/weights, fp32 PSUM accumulation (rel err ~8e-4).

Schedule (single instruction stream, deeply interleaved):
- Q streams in column chunks; attention runs per (512-query-chunk, head)
  so the first scores start after only the first Q column pass.
- Batch B's K/V/Q projections run as single-PSUM-bank steps interleaved
  into batch A's attention cadence; batch A's output projection rides
  batch B's attention; batch B's output projection is the only tail.
- PSUM: scores rotation 4x[128,512] + ctx accumulators 2 + shared aux 2
  (projection steps / out-proj) = 8 banks, one configuration end to end.
- PSUM evacuation is split across DVE and ACT (GPSIMD cannot touch PSUM);
  out-DMAs fire per query tile; PE p-state is kept warm by filler
  matmuls during the initial DMA latency.

The program is built at call time from the actual valid_lens (cached by
(KTA, KTB)); batch A is the one with more key tiles and is processed
first so its longer attention phase starts as early as possible.
"""
import sys
if "/opt/trn_rl_repo" not in sys.path:
    sys.path.insert(0, "/opt/trn_rl_repo")
import os
import time
import numpy as np

B, SQ, SK, D, H, HD = 2, 2048, 2048, 1024, 16, 64
NEG = -1.0e6
N_CORES = 8
DL = 128          # d_local: 2 heads * 64
KD = D // 128     # contraction tiles over D

_NC_CACHE = {}
last_results = None
last_exec_wall_s = None


def _build(KTA, KTB, LCA=None, LCB=None):
    import concourse.bass as bass  # noqa: F401
    import concourse.tile as tile
    from concourse import bacc, mybir

    f32 = mybir.dt.float32
    f16 = mybir.dt.float16
    EXP = mybir.ActivationFunctionType.Exp

    LKA, LKB = KTA * 128, KTB * 128
    # K/V stream DMAs only carry the valid columns (rounded up to 8);
    # the SBUF tails are zero-filled so masked tail scores stay exact
    LCA = LKA if LCA is None else LCA
    LCB = LKB if LCB is None else LCB
    # [(k0, nk)] chunk groups for the k/v/weight streams (fewer, larger DMAs)
    kgrp = [(0, 4), (4, 4)]

    nc = bacc.Bacc("TRN2", target_bir_lowering=False, debug=False,
                   num_devices=N_CORES)
    xqT = nc.dram_tensor("xqT", [2, D, SQ], f16, kind="ExternalInput")
    xkTA = nc.dram_tensor("xkTA", [D, LKA], f16, kind="ExternalInput")
    xvTA = nc.dram_tensor("xvTA", [D, LKA], f16, kind="ExternalInput")
    xkTB = nc.dram_tensor("xkTB", [D, LKB], f16, kind="ExternalInput")
    xvTB = nc.dram_tensor("xvTB", [D, LKB], f16, kind="ExternalInput")
    wqkv = nc.dram_tensor("wqkv", [D, 3 * DL], f16, kind="ExternalInput")
    wo = nc.dram_tensor("wo", [DL, D], f16, kind="ExternalInput")
    maskA = nc.dram_tensor("maskA", [128, KTA], f32, kind="ExternalInput")
    maskB = nc.dram_tensor("maskB", [128, KTB], f32, kind="ExternalInput")
    out = nc.dram_tensor("out", [2, SQ, D], f16, kind="ExternalOutput")

    with tile.TileContext(nc) as tc:
        with tc.tile_pool(name="singles", bufs=1) as sg:
            wqkv_sb = sg.tile([128, KD, 3 * DL], f16)
            wo_sb = sg.tile([128, D], f16)
            maskA_sb = sg.tile([128, KTA], f32)
            maskB_sb = sg.tile([128, KTB], f32)
            kt_sb = {0: sg.tile([128, LKA], f16, name="ktA"),
                     1: sg.tile([128, LKB], f16, name="ktB")}
            qt_sb = {0: sg.tile([128, SQ], f16, name="qtA"),
                     1: sg.tile([128, SQ], f16, name="qtB")}
            v_sb = {0: sg.tile([128, KTA, 2, 128], f16, name="vA"),
                    1: sg.tile([128, KTB, 2, 128], f16, name="vB")}
            ctx_sb = {0: sg.tile([128, SQ], f16, name="ctxA"),
                      1: sg.tile([128, SQ], f16, name="ctxB")}
            warm_sb = sg.tile([128, 256], f16)

            KT = {0: KTA, 1: KTB}
            mask_sb = {0: maskA_sb, 1: maskB_sb}
            xkT = {0: xkTA, 1: xkTB}
            xvT = {0: xvTA, 1: xvTB}

            # V'' ones-columns (softmax denominator); dim columns are
            # overwritten by the V-projection evacuations below.
            nc.gpsimd.memset(v_sb[0], 1.0)
            nc.gpsimd.memset(v_sb[1], 1.0)
            nc.vector.memset(warm_sb, 0.0)

            # ---- input DMAs, arrival order = need order ----
            nc.sync.dma_start(
                out=wqkv_sb[:, 0:4, :],
                in_=wqkv[0:512, :].rearrange("(k p) j -> p k j", p=128))
            strm_cm = tc.tile_pool(name="streams", bufs=1)
            strm = strm_cm.__enter__()
            xk = {b: strm.tile([128, KD, 128 * KT[b]], f16, name=f"xk{b}")
                  for b in (0, 1)}
            xq = {b: strm.tile([128, KD, SQ], f16, name=f"xq{b}")
                  for b in (0, 1)}
            xv = {b: strm.tile([128, KD, 128 * KT[b]], f16, name=f"xv{b}")
                  for b in (0, 1)}
            LC = {0: LCA, 1: LCB}

            def dma_kv(b, which):
                src = xkT[b] if which == "k" else xvT[b]
                dst = xk[b] if which == "k" else xv[b]
                if LC[b] < 128 * KT[b]:
                    nc.gpsimd.memset(dst[:, :, LC[b]:], 0.0)
                for k0, nk in kgrp:
                    nc.sync.dma_start(
                        out=dst[:, k0:k0 + nk, 0:LC[b]],
                        in_=src[k0 * 128:(k0 + nk) * 128, 0:LC[b]]
                        .rearrange("(k p) j -> p k j", p=128))

            def dma_q(b):
                for k in range(KD):
                    nc.sync.dma_start(out=xq[b][:, k, :],
                                      in_=xqT[b, k * 128:(k + 1) * 128, :])

            def dma_q_cols(b, first=None):
                # column-chunk order: Q-projection pass ci becomes ready
                # as soon as chunk ci lands (contraction needs all k)
                cis = range(4) if first is None else (
                    range(0, 1) if first else range(1, 4))
                for ci in cis:
                    nc.sync.dma_start(
                        out=xq[b][:, :, ci * 512:(ci + 1) * 512],
                        in_=xqT[b, :, ci * 512:(ci + 1) * 512]
                        .rearrange("(k p) j -> p k j", p=128))

            dma_kv(0, "k")
            nc.sync.dma_start(
                out=wqkv_sb[:, 4:8, :],
                in_=wqkv[512:1024, :].rearrange("(k p) j -> p k j", p=128))
            dma_kv(0, "v")   # xv before xq: ctx never stalls on V''
            dma_q_cols(0, first=1)
            # the tiny mask DMAs hold the descriptor stage for ~625ns each;
            # issued after the attention-gating Q chunk, before the first exp
            nc.sync.dma_start(out=maskA_sb, in_=maskA[:, :])
            nc.sync.dma_start(out=maskB_sb, in_=maskB[:, :])
            dma_q_cols(0, first=0)
            dma_kv(1, "k")
            dma_kv(1, "v")
            dma_q_cols(1)
            nc.sync.dma_start(out=wo_sb, in_=wo[:, :])

            # ---- PE p-state warmup: keep the tensor engine busy during the
            # initial DMA latency so real matmuls start at full clock; the
            # pool stays open so stream-gated phases can emit filler too ----
            psW_cm = tc.tile_pool(name="psW", bufs=1, space="PSUM")
            psW = psW_cm.__enter__()
            wp = psW.tile([128, 256], f32)

            def wfill(n):
                for _ in range(n):
                    nc.tensor.matmul(wp, warm_sb[:, 0:128], warm_sb,
                                     start=True, stop=True)

            wfill(22)

            def copy_eng(eng, dst, src):
                if eng == "v":
                    nc.vector.tensor_copy(dst, src)
                elif eng == "a":
                    nc.scalar.copy(dst, src)
                else:
                    nc.gpsimd.tensor_copy(dst, src)

            def proj_k(b, eng):
                # K^T[b] = Wk^T @ Xk^T : [128, LK_b]
                LK = 128 * KT[b]
                chunks = [(i * 512, min(512, LK - i * 512))
                          for i in range((LK + 511) // 512)]
                with tc.tile_pool(name=f"psA{b}", bufs=1, space="PSUM") as ps:
                    accs = [ps.tile([128, cw], f32, tag=f"kt{ci}",
                                    name=f"kt{b}_{ci}")
                            for ci, (c0, cw) in enumerate(chunks)]
                    for k in range(KD):
                        for ci, (c0, cw) in enumerate(chunks):
                            nc.tensor.matmul(accs[ci],
                                             wqkv_sb[:, k, 0:128],
                                             xk[b][:, k, c0:c0 + cw],
                                             start=(k == 0), stop=(k == KD - 1))
                    for ci, (c0, cw) in enumerate(chunks):
                        copy_eng(eng[ci % len(eng)],
                                 kt_sb[b][:, c0:c0 + cw], accs[ci])

            def proj_q(b, eng, wfill=None):
                # Q^T[b] = Wq^T @ Xq^T : [128, 2048]. wfill emits idle
                # matmuls between DMA-gated k-chunks to hold the PE p-state.
                with tc.tile_pool(name=f"psB{b}", bufs=1, space="PSUM") as ps:
                    accs = [ps.tile([128, 512], f32, tag=f"q{ci}",
                                    name=f"q{b}_{ci}") for ci in range(4)]
                    for k in range(KD):
                        for ci in range(4):
                            nc.tensor.matmul(accs[ci],
                                             wqkv_sb[:, k, 128:256],
                                             xq[b][:, k, ci * 512:(ci + 1) * 512],
                                             start=(k == 0), stop=(k == KD - 1))
                        if wfill is not None and k < KD - 1:
                            wfill(2)
                    for ci in range(4):
                        copy_eng(eng[ci % len(eng)],
                                 qt_sb[b][:, ci * 512:(ci + 1) * 512], accs[ci])

            def proj_v(b, eng):
                # V''[b] : [key, head, dim|ones]; one pool, per-group tags,
                # so a later group never waits an earlier group's evacs
                gs = [list(range(g0, min(g0 + 4, KT[b])))
                      for g0 in range(0, KT[b], 4)]
                if KT[b] > 7:   # bank budget: fall back to serial groups
                    gs = [[t] for t in range(KT[b])]
                with tc.tile_pool(name=f"psV{b}", bufs=1, space="PSUM") as ps:
                    for gi, g in enumerate(gs):
                        tag = f"vg{gi % 4}" if KT[b] > 7 else f"vg{gi}"
                        vacc = ps.tile([128, len(g), 512], f32, tag=tag,
                                       name=f"v{b}_{gi}")
                        for k in range(KD):
                            for vi, t in enumerate(g):
                                nc.tensor.matmul(
                                    vacc[:, vi, 0:DL],
                                    xv[b][:, k, t * 128:(t + 1) * 128],
                                    wqkv_sb[:, k, 256:384],
                                    start=(k == 0), stop=(k == KD - 1),
                                    skip_group_check=True)
                        for vi, t in enumerate(g):
                            for hh in range(2):
                                copy_eng(eng[(2 * vi + hh) % len(eng)],
                                         v_sb[b][:, t, hh, 0:64],
                                         vacc[:, vi, hh * 64:(hh + 1) * 64])

            def attend(b, pools, extra=None):
                # scores^T -> exp -> ctx'' per (head, 512-query-chunk,
                # key-tile), chunk-major so chunk c needs only Q column
                # pass c; ctx accumulates in a [128,512] 1-bank tile,
                # normalized per chunk. extra(i) interleaves foreign work.
                ptp, mp, psS, psC = pools
                nit = 0
                for cq in range(4):
                    for hh in range(2):
                        q0 = cq * 512
                        ctx_ps = psC.tile([128, 512], f32, tag="ctx",
                                          name=f"ctx{b}_{hh}_{cq}")
                        for t in range(KT[b]):
                            s_ps = psS.tile([128, 512], f32, tag="s")
                            nc.tensor.matmul(
                                s_ps,
                                kt_sb[b][hh * 64:hh * 64 + 64,
                                         t * 128:(t + 1) * 128],
                                qt_sb[b][hh * 64:hh * 64 + 64,
                                         q0:q0 + 512],
                                start=True, stop=True)
                            pt = ptp.tile([128, 512], f16, tag="pt")
                            nc.scalar.activation(
                                pt, s_ps, EXP,
                                bias=mask_sb[b][:, t:t + 1], scale=0.125)
                            nc.tensor.matmul(
                                ctx_ps, v_sb[b][:, t, hh, :], pt,
                                start=(t == 0), stop=(t == KT[b] - 1),
                                skip_group_check=True)
                            if extra is not None:
                                extra(nit)
                            nit += 1
                        # rows 64-127 all hold the softmax denominator
                        # (only one TensorTensor input may come from PSUM,
                        # so reciprocal lands in SBUF first)
                        rcb = mp.tile([64, 512], f32, tag="rcb")
                        nc.vector.reciprocal(rcb, ctx_ps[64:128, :])
                        nc.vector.tensor_mul(
                            ctx_sb[b][hh * 64:hh * 64 + 64, q0:q0 + 512],
                            ctx_ps[0:64, :], rcb)

            evac_ct = [0]

            def out_proj(b, pools, qr, engines):
                # partial out[b] rows = ctx''[b]^T @ Wo_local; [128,512]
                # PSUM grain; each query tile DMAs out as soon as its own
                # two evacuations land (finer DMA pipelining than pairs)
                psD, op = pools
                for qi in qr:
                    o_sb = op.tile([128, D], f16, tag=f"o{qi % 3}",
                                   name=f"osb{b}_{qi}")
                    for n in range(2):
                        o_ps = psD.tile([128, 512], f32, tag="x",
                                        name=f"o{b}_{qi}_{n}")
                        nc.tensor.matmul(o_ps,
                                         ctx_sb[b][:, qi * 128:(qi + 1) * 128],
                                         wo_sb[:, n * 512:(n + 1) * 512],
                                         start=True, stop=True)
                        eng = engines[evac_ct[0] % len(engines)]
                        evac_ct[0] += 1
                        copy_eng(eng, o_sb[:, n * 512:(n + 1) * 512], o_ps)
                    nc.sync.dma_start(
                        out=out[b, qi * 128:(qi + 1) * 128, :], in_=o_sb)

            # ---- batch A K/V projections ride the early streams; Q runs
            # as column passes so attention starts while Q still streams ----
            proj_k(0, ("v",))
            proj_v(0, ("v",))
            psW_cm.__exit__(None, None, None)

            # batch-B projections as single-PSUM-bank steps, interleaved
            # into batch-A's ACT-bound attention cadence (all on GPSIMD so
            # nothing queues behind DVE norms)
            def bsteps(psX):
                LKB_ = 128 * KT[1]

                def a1b_step(c0, cw):
                    acc = psX.tile([128, 512], f32, tag="x", name="xa")
                    for k in range(KD):
                        nc.tensor.matmul(acc[:, 0:cw], wqkv_sb[:, k, 0:128],
                                         xk[1][:, k, c0:c0 + cw],
                                         start=(k == 0), stop=(k == KD - 1))
                    copy_eng("v", kt_sb[1][:, c0:c0 + cw], acc[:, 0:cw])

                def vb_step(t):
                    acc = psX.tile([128, 512], f32, tag="x", name="xv")
                    for k in range(KD):
                        nc.tensor.matmul(acc[:, 0:DL],
                                         xv[1][:, k, t * 128:(t + 1) * 128],
                                         wqkv_sb[:, k, 256:384],
                                         start=(k == 0), stop=(k == KD - 1))
                    for hh in range(2):
                        copy_eng("v", v_sb[1][:, t, hh, 0:64],
                                 acc[:, hh * 64:(hh + 1) * 64])

                def qb_step(ci, b=1):
                    acc = psX.tile([128, 512], f32, tag="x", name="xq")
                    for k in range(KD):
                        nc.tensor.matmul(acc, wqkv_sb[:, k, 128:256],
                                         xq[b][:, k, ci * 512:(ci + 1) * 512],
                                         start=(k == 0), stop=(k == KD - 1))
                    copy_eng("v", qt_sb[b][:, ci * 512:(ci + 1) * 512], acc)

                steps = []
                for i in range((LKB_ + 511) // 512):
                    c0 = i * 512
                    steps.append(lambda c0=c0, cw=min(512, LKB_ - c0):
                                 a1b_step(c0, cw))
                steps.extend(lambda t=t: vb_step(t) for t in range(KT[1]))
                steps.extend(lambda ci=ci: qb_step(ci) for ci in range(4))
                return steps, qb_step

            # One continuous PSUM configuration from first attention to last
            # output tile: psS (2 banks, score rotation) + psC (4 banks, ctx
            # accumulator) + aux (2 banks, shared rotation for batch-B
            # projection steps, then both batches' out-projection PSUM).
            with tc.tile_pool(name="pt", bufs=6) as ptp, \
                 tc.tile_pool(name="misc", bufs=2) as mp, \
                 tc.tile_pool(name="ob", bufs=6) as op:
                with tc.tile_pool(name="psS", bufs=4, space="PSUM") as psS, \
                     tc.tile_pool(name="psC", bufs=2, space="PSUM") as psC, \
                     tc.tile_pool(name="aux", bufs=2, space="PSUM") as aux:
                    steps, qa_step = bsteps(aux)
                    # Batch A's Q column pass 0 gates the first group;
                    # passes 1-3 interleave ahead of the chunks that need
                    # them, tracking the xq column-DMA arrivals. Batch-B
                    # projection steps ride attend(0)'s second half.
                    qa_step(0, b=0)
                    nit_A = 8 * KT[0]
                    smap = {}
                    for i, frac in ((1, 0.15), (2, 0.375), (3, 0.55)):
                        it = max(i, int(nit_A * frac))
                        smap.setdefault(it, []).append(
                            lambda ci=i: qa_step(ci, b=0))
                    for j, s in enumerate(steps):
                        it = max(4, int(nit_A * (0.62 + 0.33 * j / len(steps))))
                        smap.setdefault(it, []).append(s)

                    # a small slice of batch A's out-projection (qi 0-3,
                    # query chunk 0, normalized after group 4) rides the tail
                    # of attend(0) so its out-DMAs start during the DMA lull
                    qmapA = {}

                    def extraA(i):
                        for s in smap.get(i, ()):
                            s()
                        if i in qmapA:
                            out_proj(0, (aux, op), qmapA[i], ("a", "v"))

                    attend(0, (ptp, mp, psS, psC), extra=extraA)
                    for it, fns in sorted(smap.items()):
                        if it >= nit_A:
                            for s in fns:
                                s()
                    done_A = sorted(q for qs in qmapA.values() for q in qs)
                    rest_A = [q for q in range(16) if q not in done_A]
                    # batch B attention carries the rest of batch A's
                    # out-projection, spread across its cadence
                    nb = 8 * KT[1]
                    qsched = {}
                    ns = max(1, nb - 1)
                    nr = len(rest_A)
                    for i in range(ns):
                        # later slots lean on ACT: its exps wind down while
                        # DVE still carries the trailing norms
                        eng = ("v", "v", "a") if i < ns // 2 else ("a", "v")
                        qsched[i] = (rest_A[nr * i // ns: nr * (i + 1) // ns],
                                     eng)

                    def extra(i):
                        if i in qsched:
                            qr, eng = qsched[i]
                            out_proj(0, (aux, op), qr, eng)

                    attend(1, (ptp, mp, psS, psC), extra=extra)
                # final out-projection in its own deep PSUM rotation so the
                # tail runs at the out-DMA rate, not the evacuation rate
                    # first tail tiles run from the still-open aux pool so
                    # PE flows into the tail while psD2's banks hand over
                    out_proj(1, (aux, op), range(0, 2), ("a", "v"))
                with tc.tile_pool(name="psD2", bufs=6, space="PSUM") as psD2:
                    out_proj(1, (psD2, op), range(2, 16), ("a", "v"))
            strm_cm.__exit__(None, None, None)
    nc.compile()
    return nc


def kernel(**inputs):
    global last_results, last_exec_wall_s
    from concourse.bass_utils import run_bass_kernel_spmd

    # BASS_TRACE needs the axon NTFF hook; disable tracing when the hook
    # module is unavailable so a stray env var cannot crash the run.
    if os.environ.get("BASS_TRACE"):
        try:
            from antenv import axon_hooks  # noqa: F401
        except Exception:
            os.environ["BASS_NEVER_TRACE"] = "1"

    q = np.asarray(inputs["queries"], dtype=np.float32)
    kx = np.asarray(inputs["keys"], dtype=np.float32)
    vx = np.asarray(inputs["values"], dtype=np.float32)
    vl = np.asarray(inputs["valid_lens"], dtype=np.int64).reshape(B)
    Wq = np.asarray(inputs["Wq"], dtype=np.float32)
    Wk = np.asarray(inputs["Wk"], dtype=np.float32)
    Wv = np.asarray(inputs["Wv"], dtype=np.float32)
    Wo = np.asarray(inputs["Wo"], dtype=np.float32)
    assert q.shape == (B, SQ, D) and kx.shape == (B, SK, D) and vx.shape == (B, SK, D)

    lens = np.clip(vl, 1, SK)
    KTs = [(int(l) + 127) // 128 for l in lens]
    # batch A = more key tiles, processed first
    bA = 0 if KTs[0] >= KTs[1] else 1
    bB = 1 - bA
    KTA, KTB = KTs[bA], KTs[bB]
    LKA, LKB = KTA * 128, KTB * 128

    LCA = min(LKA, -(-int(lens[bA]) // 8) * 8)
    LCB = min(LKB, -(-int(lens[bB]) // 8) * 8)
    key = (KTA, KTB, LCA, LCB)
    if key not in _NC_CACHE:
        _NC_CACHE[key] = _build(KTA, KTB, LCA, LCB)
    nc = _NC_CACHE[key]

    def m128(b, KT):
        m = np.where(np.arange(KT * 128) < lens[b], 0.0, NEG).astype(np.float32)
        return np.ascontiguousarray(m.reshape(KT, 128).T)

    xqT_full = np.ascontiguousarray(
        np.stack([q[bA].T, q[bB].T]).astype(np.float16))
    in_maps = []
    for c in range(N_CORES):
        cols = slice(DL * c, DL * (c + 1))
        in_maps.append({
            "xqT": xqT_full,
            "xkTA": np.ascontiguousarray(kx[bA, :LKA].T.astype(np.float16)),
            "xvTA": np.ascontiguousarray(vx[bA, :LKA].T.astype(np.float16)),
            "xkTB": np.ascontiguousarray(kx[bB, :LKB].T.astype(np.float16)),
            "xvTB": np.ascontiguousarray(vx[bB, :LKB].T.astype(np.float16)),
            "wqkv": np.ascontiguousarray(np.concatenate(
                [Wk[:, cols], Wq[:, cols], Wv[:, cols]],
                axis=1).astype(np.float16)),
            "wo": np.ascontiguousarray(Wo[cols, :].astype(np.float16)),
            "maskA": m128(bA, KTA),
            "maskB": m128(bB, KTB),
        })

    t0 = time.perf_counter()
    res = run_bass_kernel_spmd(nc, in_maps, core_ids=list(range(N_CORES)))
    last_exec_wall_s = time.perf_counter() - t0
    last_results = res

    outs = [res.results[c]["out"].astype(np.float32) for c in range(N_CORES)]
    acc = outs[0]
    for c in range(1, N_CORES):
        acc = acc + outs[c]
    full = np.empty((B, SQ, D), dtype=np.float32)
    full[bA] = acc[0]
    full[bB] = acc[1]
    return full
